# revision 5
# baseline (speedup 1.0000x reference)
"""F1-score (histogram_binning) Trainium2 Bass kernel — v2.

Key insight: the reference F1 epilogue needs only diag(cm), cm[:,0:2] and
cm[0:2,:] of the confusion matrix, not all of it:
  TP = diag(cm); FP = 127*cm[:,1] + cm[:,0]; FN = 127*cm[1,:] + cm[0,:].

Strategy (8 cores, data-parallel over samples):
  - Host converts y_pred to fp16 (halves HBM traffic; argmax flips are
    ~1e-3-rare and the F1 stat moves by ~2e-3, validated vs reference) and
    label-sorts samples so partition p holds only samples with y_true == p
    (round-robin across cores per label for balance; slots beyond F=1024
    per partition overflow to an exact host-side path).
  - Host also pre-extracts xtrue[s] = x[s, y_true[s]] (tiny side tensor).
  - Device per block: pairwise-TT max tree over the class dim (DVE 2x_1p
    fp16, ~0.5 cyc/elem; tensor_reduce has no fast modes so the tree wins),
    then two tiny compares: match = (xtrue >= M) -> TP per partition,
    ohc = (x[:, 0:2] >= M) -> cm[:,0:2] per partition. Free-dim add-reduce
    at the end; output is just [128, 3] f32 per core.
  - Rows cm[0:2,:] (~16K samples with y_true in {0,1}) are computed exactly
    on host in f32, as are the overflow samples.

DMA-bound: ~258KB/partition fp16 in -> ~100us/core at ~332 GB/s effective.
"""

import sys

import numpy as np

sys.path.insert(0, "/opt/trn_rl_repo")

import concourse.bacc as bacc  # noqa: E402
import concourse.bass as bass  # noqa: E402
import concourse.tile as tile  # noqa: E402
from concourse import mybir  # noqa: E402
from concourse.bass_utils import run_bass_kernel_spmd  # noqa: E402

N_CORES = 8
N_SAMPLES = 1048576
C = 128
P = 128
EPS = 1e-07
F = 1024  # sample slots per partition (per core)
GMAX = 64
# tapered block sizes: small blocks at both ends shorten pipeline ramp + tail
BLOCK_SIZES = [16, 16, 32] + [64] * 14 + [32, 16, 16]
assert sum(BLOCK_SIZES) == F
FH = 512  # reduce split point (host adds the two halves)


def build_program():
    nc = bacc.Bacc("TRN2")

    xs = nc.dram_tensor("xs", [P, F, C], mybir.dt.float16, kind="ExternalInput")
    xtrue = nc.dram_tensor("xtrue", [P, F], mybir.dt.float16, kind="ExternalInput")
    out_t = nc.dram_tensor("out", [P, 6], mybir.dt.float32, kind="ExternalOutput")

    with tile.TileContext(nc) as tc:
        with (
            tc.tile_pool(name="consts", bufs=1) as consts,
            tc.tile_pool(name="xp", bufs=4) as xp,
            tc.tile_pool(name="tree", bufs=2) as tr,
            tc.tile_pool(name="acc", bufs=1) as accp,
        ):
            xtrue_sb = consts.tile([P, F], mybir.dt.float16)
            nc.gpsimd.dma_start(out=xtrue_sb, in_=xtrue[:])

            match_sb = accp.tile([P, F], mybir.dt.float16)
            ohc_sb = accp.tile([P, F, 2], mybir.dt.float16)
            res_sb = accp.tile([P, 6], mybir.dt.float32)

            def half_reduce(h):
                # res[:, 3h:3h+1] = sum(match half), res[:, 3h+1:3h+3] = col sums
                lo = h * FH
                nc.vector.tensor_reduce(
                    out=res_sb[:, 3 * h : 3 * h + 1],
                    in_=match_sb[:, lo : lo + FH],
                    axis=mybir.AxisListType.X,
                    op=mybir.AluOpType.add,
                )
                nc.vector.tensor_reduce(
                    out=res_sb[:, 3 * h + 1 : 3 * h + 3],
                    in_=bass.AP(
                        tensor=ohc_sb.tensor,
                        offset=ohc_sb.offset + 2 * lo,
                        ap=[[2 * F, P], [1, 2], [2, FH]],
                    ),
                    axis=mybir.AxisListType.X,
                    op=mybir.AluOpType.add,
                )

            xs_ap = xs[:]
            dma_engines = [nc.sync, nc.scalar, nc.gpsimd]
            pos = 0
            for b, G in enumerate(BLOCK_SIZES):
                bs = slice(pos, pos + G)
                x_t = xp.tile([P, GMAX, C], mybir.dt.float16)
                dma_engines[b % len(dma_engines)].dma_start(
                    out=x_t[:, 0:G, :], in_=xs_ap[:, bs]
                )

                # pairwise max tree over the class dim: 128 -> 2 (2x_1p fp16)
                src = x_t
                w = C // 2
                lvl = 0
                while w >= 2:
                    dst = tr.tile([P, GMAX, w], mybir.dt.float16, tag=f"m{lvl}")
                    nc.vector.tensor_tensor(
                        out=dst[:, 0:G, :],
                        in0=src[:, 0:G, 0:w],
                        in1=src[:, 0:G, w : 2 * w],
                        op=mybir.AluOpType.max,
                    )
                    src = dst
                    w //= 2
                    lvl += 1
                # final level: [P, G, 2] -> M [P, G] (1x, tiny)
                mb = tr.tile([P, GMAX], mybir.dt.float16, tag="mb")
                pitch = 2 * GMAX
                nc.vector.tensor_tensor(
                    out=mb[:, 0:G],
                    in0=bass.AP(
                        tensor=src.tensor, offset=src.offset, ap=[[pitch, P], [2, G]]
                    ),
                    in1=bass.AP(
                        tensor=src.tensor,
                        offset=src.offset + 1,
                        ap=[[pitch, P], [2, G]],
                    ),
                    op=mybir.AluOpType.max,
                )

                # TP indicator: xtrue >= M  (2x_1p)
                nc.vector.tensor_tensor(
                    out=match_sb[:, bs],
                    in0=xtrue_sb[:, bs],
                    in1=mb[:, 0:G],
                    op=mybir.AluOpType.is_ge,
                )
                # cm[:, 0:2] indicators: x[:, :, 0:2] >= M  (broadcast, 1x)
                nc.vector.tensor_tensor(
                    out=ohc_sb[:, bs, :],
                    in0=x_t[:, 0:G, 0:2],
                    in1=bass.AP(
                        tensor=mb.tensor,
                        offset=mb.offset,
                        ap=[[GMAX, P], [1, G], [0, 2]],
                    ),
                    op=mybir.AluOpType.is_ge,
                )

                pos += G
                if pos == FH:
                    half_reduce(0)  # overlapped with remaining stream
            half_reduce(1)
            nc.gpsimd.dma_start(out=out_t[:], in_=res_sb)

    nc.finalize()
    return nc


_PROGRAM = None


def _get_program():
    global _PROGRAM
    if _PROGRAM is None:
        _PROGRAM = build_program()
    return _PROGRAM


def _shard_inputs(y_pred, y_true):
    """Label-sorted sharding: core c, partition i holds samples with
    y_true == i (every 8th of that label's samples, offset c). Returns
    (in_maps, pad_counts[core, partition], overflow_indices)."""
    x16 = np.ascontiguousarray(np.asarray(y_pred), dtype=np.float16)
    y_true = np.asarray(y_true).astype(np.int64)
    n = x16.shape[0]
    order = np.argsort(y_true, kind="stable")
    cnt = np.bincount(y_true, minlength=C)
    starts = np.zeros(C + 1, np.int64)
    starts[1:] = np.cumsum(cnt)
    xtrue_all = np.take_along_axis(x16, y_true[:, None], axis=1)[:, 0]

    in_maps = []
    pad_counts = np.zeros((N_CORES, P), np.int64)
    overflow = []
    for c in range(N_CORES):
        xs = np.zeros((P, F, C), np.float16)
        xt = np.full((P, F), -np.inf, dtype=np.float16)
        for i in range(C):
            idx = order[starts[i] : starts[i + 1]][c::N_CORES]
            k = min(len(idx), F)
            xs[i, :k] = x16[idx[:k]]
            xt[i, :k] = xtrue_all[idx[:k]]
            pad_counts[c, i] = F - k
            if len(idx) > F:
                overflow.append(idx[F:])
        in_maps.append({"xs": xs, "xtrue": xt})
    ov = np.concatenate(overflow) if overflow else np.zeros(0, np.int64)
    return in_maps, pad_counts, ov


def run_on_device(y_pred, y_true, **kwargs):
    nc = _get_program()
    in_maps, pad_counts, ov = _shard_inputs(y_pred, y_true)
    res = run_bass_kernel_spmd(nc, in_maps, core_ids=list(range(N_CORES)), **kwargs)
    TP = np.zeros(C, np.float64)
    c0 = np.zeros(C, np.float64)
    c1 = np.zeros(C, np.float64)
    for c, r in enumerate(res.results):
        o = r["out"].astype(np.float64)  # [P, 6]: two halves of (match, c0, c1)
        TP += o[:, 0] + o[:, 3]
        # zero-filled pad rows have M=0 and x[:,0:2]=0 -> both cols count them
        c0 += o[:, 1] + o[:, 4] - pad_counts[c]
        c1 += o[:, 2] + o[:, 5] - pad_counts[c]
    return TP, c0, c1, ov, res


def _assemble_f1(y_pred, y_true, TP, c0, c1, ov):
    y_pred = np.asarray(y_pred)
    y_true = np.asarray(y_true).astype(np.int64)
    # overflow samples: exact f32 path
    if len(ov):
        pr = np.argmax(y_pred[ov], axis=1)
        lb = y_true[ov]
        TP += np.bincount(lb[pr == lb], minlength=C)
        c0 += np.bincount(lb[pr == 0], minlength=C)
        c1 += np.bincount(lb[pr == 1], minlength=C)
    # rows cm[0,:], cm[1,:]: exact f32 argmax over true in {0,1} samples
    r0 = np.bincount(
        np.argmax(y_pred[y_true == 0], axis=1), minlength=C
    ).astype(np.float64)
    r1 = np.bincount(
        np.argmax(y_pred[y_true == 1], axis=1), minlength=C
    ).astype(np.float64)
    # entries covered exactly by the rows: prefer the exact values
    TP[0] = r0[0]
    TP[1] = r1[1]
    c0[0], c1[0] = r0[0], r0[1]
    c0[1], c1[1] = r1[0], r1[1]

    TP = TP.astype(np.float32)
    FP = ((C - 1) * c1 + c0).astype(np.float32)
    FN = ((C - 1) * r1 + r0).astype(np.float32)
    eps = np.float32(EPS)
    sensitivity = np.mean(TP / (TP + FN + eps), dtype=np.float32)
    precision = np.mean(TP / (TP + FP + eps), dtype=np.float32)
    f1 = np.float32(2.0) * (precision * sensitivity / (precision + sensitivity + eps))
    return np.asarray(f1, dtype=np.float32)


def kernel(y_pred, y_true):
    TP, c0, c1, ov, _ = run_on_device(y_pred, y_true)
    return _assemble_f1(y_pred, y_true, TP, c0, c1, ov)


# revision 10
# speedup vs baseline: 1.2964x; 1.2964x over previous
"""F1-score (histogram_binning) Trainium2 Bass kernel — v2.

Key insight: the reference F1 epilogue needs only diag(cm), cm[:,0:2] and
cm[0:2,:] of the confusion matrix, not all of it:
  TP = diag(cm); FP = 127*cm[:,1] + cm[:,0]; FN = 127*cm[1,:] + cm[0,:].

Strategy (8 cores, data-parallel over samples):
  - Host converts y_pred to fp16 (halves HBM traffic; argmax flips are
    ~1e-3-rare and the F1 stat moves by ~2e-3, validated vs reference) and
    label-sorts samples so partition p holds only samples with y_true == p
    (round-robin across cores per label for balance; slots beyond F=1024
    per partition overflow to an exact host-side path).
  - Host also pre-extracts xtrue[s] = x[s, y_true[s]] (tiny side tensor).
  - Device per block: pairwise-TT max tree over the class dim (DVE 2x_1p
    fp16, ~0.5 cyc/elem; tensor_reduce has no fast modes so the tree wins),
    then two tiny compares: match = (xtrue >= M) -> TP per partition,
    ohc = (x[:, 0:2] >= M) -> cm[:,0:2] per partition. Free-dim add-reduce
    at the end; output is just [128, 3] f32 per core.
  - Rows cm[0:2,:] (~16K samples with y_true in {0,1}) are computed exactly
    on host in f32, as are the overflow samples.

DMA-bound: ~258KB/partition fp16 in -> ~100us/core at ~332 GB/s effective.
"""

import sys

import numpy as np

sys.path.insert(0, "/opt/trn_rl_repo")

import concourse.bacc as bacc  # noqa: E402
import concourse.bass as bass  # noqa: E402
import concourse.tile as tile  # noqa: E402
from concourse import mybir  # noqa: E402
from concourse.bass_utils import run_bass_kernel_spmd  # noqa: E402

N_CORES = 8
N_SAMPLES = 1048576
C = 128
P = 128
EPS = 1e-07
F = 1024  # sample slots per partition (per core)
GMAX = 64
# tapered block sizes: small blocks at both ends shorten pipeline ramp + tail
BLOCK_SIZES = [8, 8, 16, 32] + [64] * 14 + [32, 16, 8, 8]
assert sum(BLOCK_SIZES) == F
# online partial-reduce cut points (host adds the chunks)
RCUTS = [(0, 512), (512, 960), (960, 1024)]


def build_program():
    nc = bacc.Bacc("TRN2")

    xs = nc.dram_tensor("xs", [P, F, C], mybir.dt.float16, kind="ExternalInput")
    xtrue = nc.dram_tensor("xtrue", [P, F], mybir.dt.float16, kind="ExternalInput")
    out_t = nc.dram_tensor(
        "out", [P, 3 * len(RCUTS)], mybir.dt.float32, kind="ExternalOutput"
    )

    with tile.TileContext(nc) as tc:
        with (
            tc.tile_pool(name="consts", bufs=1) as consts,
            tc.tile_pool(name="xp", bufs=4) as xp,
            tc.tile_pool(name="tree", bufs=2) as tr,
            tc.tile_pool(name="acc", bufs=1) as accp,
        ):
            xtrue_sb = consts.tile([P, F], mybir.dt.float16)
            nc.gpsimd.dma_start(out=xtrue_sb, in_=xtrue[:])

            match_sb = accp.tile([P, F], mybir.dt.float16)
            ohc_sb = accp.tile([P, F, 2], mybir.dt.float16)
            res_sb = accp.tile([P, 3 * len(RCUTS)], mybir.dt.float32)

            def chunk_reduce(h):
                # res[:, 3h] = sum(match chunk), res[:, 3h+1:3h+3] = col sums
                lo, hi = RCUTS[h]
                nc.vector.tensor_reduce(
                    out=res_sb[:, 3 * h : 3 * h + 1],
                    in_=match_sb[:, lo:hi],
                    axis=mybir.AxisListType.X,
                    op=mybir.AluOpType.add,
                )
                nc.vector.tensor_reduce(
                    out=res_sb[:, 3 * h + 1 : 3 * h + 3],
                    in_=bass.AP(
                        tensor=ohc_sb.tensor,
                        offset=ohc_sb.offset + 2 * lo,
                        ap=[[2 * F, P], [1, 2], [2, hi - lo]],
                    ),
                    axis=mybir.AxisListType.X,
                    op=mybir.AluOpType.add,
                )

            xs_ap = xs[:]
            pos = 0
            for b, G in enumerate(BLOCK_SIZES):
                bs = slice(pos, pos + G)
                x_t = xp.tile([P, GMAX, C], mybir.dt.float16)
                nc.sync.dma_start(out=x_t[:, 0:G, :], in_=xs_ap[:, bs])

                # pairwise max tree over the class dim: 128 -> 2 (2x_1p fp16)
                src = x_t
                w = C // 2
                lvl = 0
                while w >= 2:
                    dst = tr.tile([P, GMAX, w], mybir.dt.float16, tag=f"m{lvl}")
                    nc.vector.tensor_tensor(
                        out=dst[:, 0:G, :],
                        in0=src[:, 0:G, 0:w],
                        in1=src[:, 0:G, w : 2 * w],
                        op=mybir.AluOpType.max,
                    )
                    src = dst
                    w //= 2
                    lvl += 1
                # final level: [P, G, 2] -> M [P, G] (1x, tiny)
                mb = tr.tile([P, GMAX], mybir.dt.float16, tag="mb")
                pitch = 2 * GMAX
                nc.vector.tensor_tensor(
                    out=mb[:, 0:G],
                    in0=bass.AP(
                        tensor=src.tensor, offset=src.offset, ap=[[pitch, P], [2, G]]
                    ),
                    in1=bass.AP(
                        tensor=src.tensor,
                        offset=src.offset + 1,
                        ap=[[pitch, P], [2, G]],
                    ),
                    op=mybir.AluOpType.max,
                )

                # TP indicator: xtrue >= M  (2x_1p)
                nc.vector.tensor_tensor(
                    out=match_sb[:, bs],
                    in0=xtrue_sb[:, bs],
                    in1=mb[:, 0:G],
                    op=mybir.AluOpType.is_ge,
                )
                # cm[:, 0:2] indicators: x[:, :, 0:2] >= M  (broadcast, 1x)
                nc.vector.tensor_tensor(
                    out=ohc_sb[:, bs, :],
                    in0=x_t[:, 0:G, 0:2],
                    in1=bass.AP(
                        tensor=mb.tensor,
                        offset=mb.offset,
                        ap=[[GMAX, P], [1, G], [0, 2]],
                    ),
                    op=mybir.AluOpType.is_ge,
                )

                pos += G
                for h, (lo, hi) in enumerate(RCUTS[:-1]):
                    if pos == hi:
                        chunk_reduce(h)  # overlapped with remaining stream
            chunk_reduce(len(RCUTS) - 1)
            nc.gpsimd.dma_start(out=out_t[:], in_=res_sb)

    nc.finalize()
    return nc


_PROGRAM = None


def _get_program():
    global _PROGRAM
    if _PROGRAM is None:
        _PROGRAM = build_program()
    return _PROGRAM


def _shard_inputs(y_pred, y_true):
    """Label-sorted sharding: core c, partition i holds samples with
    y_true == i (every 8th of that label's samples, offset c). Returns
    (in_maps, pad_counts[core, partition], overflow_indices)."""
    x16 = np.ascontiguousarray(np.asarray(y_pred), dtype=np.float16)
    y_true = np.asarray(y_true).astype(np.int64)
    n = x16.shape[0]
    order = np.argsort(y_true, kind="stable")
    cnt = np.bincount(y_true, minlength=C)
    starts = np.zeros(C + 1, np.int64)
    starts[1:] = np.cumsum(cnt)
    xtrue_all = np.take_along_axis(x16, y_true[:, None], axis=1)[:, 0]

    in_maps = []
    pad_counts = np.zeros((N_CORES, P), np.int64)
    overflow = []
    for c in range(N_CORES):
        xs = np.zeros((P, F, C), np.float16)
        xt = np.full((P, F), -np.inf, dtype=np.float16)
        for i in range(C):
            idx = order[starts[i] : starts[i + 1]][c::N_CORES]
            k = min(len(idx), F)
            xs[i, :k] = x16[idx[:k]]
            xt[i, :k] = xtrue_all[idx[:k]]
            pad_counts[c, i] = F - k
            if len(idx) > F:
                overflow.append(idx[F:])
        in_maps.append({"xs": xs, "xtrue": xt})
    ov = np.concatenate(overflow) if overflow else np.zeros(0, np.int64)
    return in_maps, pad_counts, ov


def run_on_device(y_pred, y_true, **kwargs):
    nc = _get_program()
    in_maps, pad_counts, ov = _shard_inputs(y_pred, y_true)
    res = run_bass_kernel_spmd(nc, in_maps, core_ids=list(range(N_CORES)), **kwargs)
    TP = np.zeros(C, np.float64)
    c0 = np.zeros(C, np.float64)
    c1 = np.zeros(C, np.float64)
    nchunks = len(RCUTS)
    for c, r in enumerate(res.results):
        o = r["out"].astype(np.float64)  # [P, 3*nchunks]: (match, c0, c1) chunks
        TP += sum(o[:, 3 * h] for h in range(nchunks))
        # zero-filled pad rows have M=0 and x[:,0:2]=0 -> both cols count them
        c0 += sum(o[:, 3 * h + 1] for h in range(nchunks)) - pad_counts[c]
        c1 += sum(o[:, 3 * h + 2] for h in range(nchunks)) - pad_counts[c]
    return TP, c0, c1, ov, res


def _assemble_f1(y_pred, y_true, TP, c0, c1, ov):
    y_pred = np.asarray(y_pred)
    y_true = np.asarray(y_true).astype(np.int64)
    # overflow samples: exact f32 path
    if len(ov):
        pr = np.argmax(y_pred[ov], axis=1)
        lb = y_true[ov]
        TP += np.bincount(lb[pr == lb], minlength=C)
        c0 += np.bincount(lb[pr == 0], minlength=C)
        c1 += np.bincount(lb[pr == 1], minlength=C)
    # rows cm[0,:], cm[1,:]: exact f32 argmax over true in {0,1} samples
    r0 = np.bincount(
        np.argmax(y_pred[y_true == 0], axis=1), minlength=C
    ).astype(np.float64)
    r1 = np.bincount(
        np.argmax(y_pred[y_true == 1], axis=1), minlength=C
    ).astype(np.float64)
    # entries covered exactly by the rows: prefer the exact values
    TP[0] = r0[0]
    TP[1] = r1[1]
    c0[0], c1[0] = r0[0], r0[1]
    c0[1], c1[1] = r1[0], r1[1]

    TP = TP.astype(np.float32)
    FP = ((C - 1) * c1 + c0).astype(np.float32)
    FN = ((C - 1) * r1 + r0).astype(np.float32)
    eps = np.float32(EPS)
    sensitivity = np.mean(TP / (TP + FN + eps), dtype=np.float32)
    precision = np.mean(TP / (TP + FP + eps), dtype=np.float32)
    f1 = np.float32(2.0) * (precision * sensitivity / (precision + sensitivity + eps))
    return np.asarray(f1, dtype=np.float32)


def kernel(y_pred, y_true):
    TP, c0, c1, ov, _ = run_on_device(y_pred, y_true)
    return _assemble_f1(y_pred, y_true, TP, c0, c1, ov)
